# revision 18
# baseline (speedup 1.0000x reference)
"""Two-layer GraphSAGE (mean aggregation) on 8 Trainium2 NeuronCores.

Strategy (dst-partitioned, graph/data parallel):
- Nodes partitioned by destination across 8 cores (12500 each, padded to
  12544 = 98*128). The padded global row space is [half, core, 6272]; the
  between-layer AllGather is two half-collectives whose outputs are the four
  25088-row gather tables (quarter q = (half q//2, core group q%2), all
  int16-addressable).
- Edges bucketed per core by (group of 14 dst tiles, src quarter), sorted by
  dst tile. Source rows are fetched with large SWDGE dma_gather calls spread
  over the 4 SWDGE queues (queue q -> Q7 core pair 2q/2q+1) so descriptor
  generation proceeds 4-way parallel. Slot layout is identical on every core
  (capacity = max count over cores; pad slots gather row 0 with a zero
  indicator row) keeping the SPMD program uniform.
- Aggregation: 0/1 indicator tiles (slot -> dst one-hots) are host data,
  precomputed in fp8e4m3 and DMA'd in (HWDGE), used directly as the moving
  operand of the PE accumulation matmuls (bf16 lhsT x fp8 rhs). Indicators
  are A/B-split (A = src half 0, B = half 1) so layer 2 runs an X phase
  (A-side partial aggregation after the first half-AllGather) and a Y phase
  (B side + output) gated only on the second half-AllGather.
- Mean normalization is a single DVE multiply with a resident recip-broadcast
  tile; h = relu(aggTs @ W_l + selfT @ W_r) accumulates in one PSUM group.
  No dma_start_transpose anywhere (it serializes against collectives): x
  arrives host-transposed, layer 1 emits h^T via a second PSUM chain
  (W^T @ aggTs), and layer 2 produces the output transposed ([feat, node]);
  the host transposes it back. h/h^T/out DMA writes are batched per block.

kernel(**inputs) -> np.ndarray takes FULL inputs, returns FULL [100000, 128]
float32 output; all sharding happens inside.
"""

import numpy as np

P = 128
NCORES = 8
NPC = 12500
TPC = 98
NPC_PAD = TPC * P            # 12544
NALL = NCORES * NPC_PAD      # 100352
NQ = 4                       # src quarters (= gather tables)
HROWS = NPC_PAD // 2         # 6272 rows per half per core
QROWS = NALL // NQ           # 25088 rows per gather table (< int16 range)
TG = 14                      # dst tiles per group
NG = TPC // TG               # 7 groups (= gather blocks)


def _prep(edge_index):
    src = edge_index[0].astype(np.int64)
    dst = edge_index[1].astype(np.int64)
    core = dst // NPC
    loc = dst % NPC
    tl = loc // P
    off = loc % P
    g = tl // TG
    tl_loc = tl % TG
    # padded global row space: [half, core, HROWS]
    sloc = src % NPC
    half = sloc // HROWS
    srcpad = half * (NCORES * HROWS) + (src // NPC) * HROWS + sloc % HROWS
    q = srcpad // QROWS
    qrow = srcpad % QROWS

    key = ((core * NG + g) * NQ + q) * TG + tl_loc
    cnt = np.bincount(key, minlength=NCORES * NG * NQ * TG).reshape(
        NCORES, NG, NQ, TG
    )
    cap = cnt.max(axis=0)
    scum = np.zeros((NG, NQ, TG + 1), np.int64)
    np.cumsum(cap, axis=2, out=scum[:, :, 1:])
    segcols = -(-scum[:, :, TG] // P)

    # one gather call per (block=group, quarter); columns packed q-major
    colbase = np.zeros((NG, NQ), np.int64)
    calls = []                                  # (g, q, col0, callcols)
    ncols = 0
    for gg in range(NG):
        for qq in range(NQ):
            colbase[gg, qq] = ncols
            ncols += int(segcols[gg, qq])
            calls.append((gg, qq, int(colbase[gg, qq]), int(segcols[gg, qq])))

    order = np.lexsort((tl_loc, q, g, core))
    sk = key[order]
    first = np.r_[True, sk[1:] != sk[:-1]]
    idx_of_first = np.where(first)[0]
    grp_id = np.cumsum(first) - 1
    rank = np.arange(len(sk)) - idx_of_first[grp_id]
    go, qo, to, co = g[order], q[order], tl_loc[order], core[order]
    slot = scum[go, qo, to] + rank
    gcol = colbase[go, qo] + slot // P
    prow = slot % P

    cnt_dst = np.bincount(dst, minlength=NCORES * NPC).astype(np.float64)
    recip_dst = (1.0 / np.maximum(cnt_dst, 1.0)).astype(np.float32)

    idx16 = np.zeros((NCORES, 16, ncols * 8), np.int16)
    callslot = (gcol - colbase[go, qo]) * P + prow
    callb = colbase[go, qo] * 8
    idx16[co, callslot % 16, callb + callslot // 16] = qrow[order].astype(
        np.int16
    )
    idx16 = np.ascontiguousarray(np.tile(idx16, (1, 8, 1)))

    # incidence lists: per (g, t) the columns intersecting tile t, split into
    # A (src half 0: quarters 0-1) and B (half 1: quarters 2-3)
    incA = [[[] for _ in range(TG)] for _ in range(NG)]
    incB = [[[] for _ in range(TG)] for _ in range(NG)]
    posA = np.full((NG, TG, ncols), -1, np.int64)
    posB = np.full((NG, TG, ncols), -1, np.int64)
    nA = nB = 0
    for gg in range(NG):
        for t in range(TG):
            for qq in range(NQ):
                c = cap[gg, qq, t]
                if c == 0:
                    continue
                s0 = scum[gg, qq, t]
                for cc in range(s0 // P, -(-(s0 + c) // P)):
                    col = int(colbase[gg, qq] + cc)
                    if qq < 2:
                        if posA[gg, t, col] < 0:
                            posA[gg, t, col] = nA
                            incA[gg][t].append(col)
                            nA += 1
                    else:
                        if posB[gg, t, col] < 0:
                            posB[gg, t, col] = nB
                            incB[gg][t].append(col)
                            nB += 1

    import ml_dtypes

    f8 = ml_dtypes.float8_e4m3
    indA = np.zeros((NCORES, P, nA * P), f8)
    indB = np.zeros((NCORES, P, nB * P), f8)
    oo = off[order]
    isA = qo < 2
    pA = posA[go[isA], to[isA], gcol[isA]]
    assert (pA >= 0).all()
    indA[co[isA], prow[isA], pA * P + oo[isA]] = 1.0
    isB = ~isA
    pB = posB[go[isB], to[isB], gcol[isB]]
    assert (pB >= 0).all()
    indB[co[isB], prow[isB], pB * P + oo[isB]] = 1.0

    # recip broadcast tiles: [P, NPC_PAD] bf16, every partition row equal
    bf = ml_dtypes.bfloat16
    rb = np.zeros((NCORES, P, NPC_PAD), bf)
    for c in range(NCORES):
        r = np.zeros(NPC_PAD, np.float32)
        r[:NPC] = recip_dst[c * NPC : (c + 1) * NPC]
        rb[c] = np.broadcast_to(r.astype(bf), (P, NPC_PAD))

    # per-group offsets into the A/B indicator tensors
    goffA = [min((posA[gg, t, c] for t in range(TG) for c in incA[gg][t]),
                 default=0) for gg in range(NG)]
    gincA = [sum(len(incA[gg][t]) for t in range(TG)) for gg in range(NG)]
    goffB = [min((posB[gg, t, c] for t in range(TG) for c in incB[gg][t]),
                 default=0) for gg in range(NG)]
    gincB = [sum(len(incB[gg][t]) for t in range(TG)) for gg in range(NG)]

    halfcols = {}
    for gg in range(NG):
        for qq in range(NQ):
            halfcols[(gg, qq)] = int(-(-scum[gg, qq, TG // 2] // P))

    meta = dict(
        halfcols=halfcols,
        calls=calls, ncols=ncols,
        incA=incA, incB=incB, posA=posA, posB=posB,
        goffA=goffA, gincA=gincA, goffB=goffB, gincB=gincB,
        nA=nA, nB=nB,
    )
    return rb, idx16, indA, indB, meta


def _build(meta, skip_bias):
    from concourse import bacc, bass, mybir, tile

    bf16 = mybir.dt.bfloat16
    f8 = mybir.dt.float8e4
    f32 = mybir.dt.float32
    i16 = mybir.dt.int16

    calls = meta["calls"]
    halfcols = meta["halfcols"]
    ncols = meta["ncols"]
    incA, incB = meta["incA"], meta["incB"]
    posA, posB = meta["posA"], meta["posB"]
    goffA, gincA = meta["goffA"], meta["gincA"]
    goffB, gincB = meta["goffB"], meta["gincB"]
    nA, nB = meta["nA"], meta["nB"]

    nc = bacc.Bacc(
        "TRN2", target_bir_lowering=False, debug=False, num_devices=NCORES,
        num_swdge_queues=4,
    )

    xh = [
        nc.declare_dram_parameter(f"xh{i}", [2 * QROWS, P], bf16,
                                  isOutput=False)
        for i in range(2)
    ]
    xownT = nc.declare_dram_parameter("xownT", [P, NPC_PAD], bf16, isOutput=False)
    idx_d = nc.declare_dram_parameter("idx16", [P, ncols * 8], i16, isOutput=False)
    indA_d = nc.declare_dram_parameter("indA", [P, nA * P], f8, isOutput=False)
    indB_d = nc.declare_dram_parameter("indB", [P, nB * P], f8, isOutput=False)
    rb_d = nc.declare_dram_parameter("rb", [P, NPC_PAD], bf16, isOutput=False)
    wl1_d = nc.declare_dram_parameter("wl1", [P, P], bf16, isOutput=False)
    wr1_d = nc.declare_dram_parameter("wr1", [P, P], bf16, isOutput=False)
    wl2_d = nc.declare_dram_parameter("wl2", [P, P], bf16, isOutput=False)
    wr2_d = nc.declare_dram_parameter("wr2", [P, P], bf16, isOutput=False)
    out_d = nc.declare_dram_parameter("out", [P, NPC_PAD], bf16, isOutput=True)

    maxcallcols = max(c[3] for c in calls)
    maxgA = max(gincA) if max(gincA) else 1
    maxgB = max(gincB) if max(gincB) else 1
    call_of = {}
    for (gg, qq, c0, ccols) in calls:
        call_of[(gg, qq)] = (c0, ccols)

    with tile.TileContext(nc) as tc:
        with (
            tc.tile_pool(name="const", bufs=1) as cpool,
            tc.tile_pool(name="gath", bufs=2) as gpool,
            tc.tile_pool(name="indp", bufs=2) as ipool,
            tc.tile_pool(name="xot", bufs=3) as tpool,
            tc.tile_pool(name="aggp", bufs=8) as apool,
            tc.tile_pool(name="aggA", bufs=1) as aApool,
            tc.tile_pool(name="stg", bufs=2) as stgpool,
            tc.tile_pool(name="psacc", bufs=4, space="PSUM") as ps_acc,
            tc.tile_pool(name="psh", bufs=2, space="PSUM") as ps_h,
            tc.tile_pool(name="psht", bufs=2, space="PSUM") as ps_hT,
            tc.tile_pool(name="dram", bufs=1, space="DRAM") as dpool,
        ):
            def cload(dram_ap, shape, dtype, name):
                t = cpool.tile(shape, dtype, name=name)
                nc.sync.dma_start(out=t[:], in_=dram_ap)
                return t

            idx_sb = cload(idx_d[:], [P, ncols * 8], i16, "idx16")
            wl1 = cload(wl1_d[:], [P, P], bf16, "wl1")
            wr1 = cload(wr1_d[:], [P, P], bf16, "wr1")
            wl2 = cload(wl2_d[:], [P, P], bf16, "wl2")
            rbc = cload(rb_d[:], [P, NPC_PAD], bf16, "rb")
            wr2 = cload(wr2_d[:], [P, P], bf16, "wr2")

            h_bounce = dpool.tile([NPC_PAD, P], bf16, name="h_bounce")
            hT_bounce = dpool.tile([P, NPC_PAD], bf16, name="hT_bounce")
            h_half = [
                dpool.tile([2 * QROWS, P], bf16, name=f"h_half{i}",
                           addr_space="Shared")
                for i in range(2)
            ]
            xq_aps = [
                xh[0][0:QROWS, :], xh[0][QROWS : 2 * QROWS, :],
                xh[1][0:QROWS, :], xh[1][QROWS : 2 * QROWS, :],
            ]
            hq_aps = [
                h_half[0][0:QROWS, :], h_half[0][QROWS : 2 * QROWS, :],
                h_half[1][0:QROWS, :], h_half[1][QROWS : 2 * QROWS, :],
            ]

            def gather(b, qq, queue, qtab):
                c0, ccols = call_of[(b, qq)]
                if ccols == 0:
                    return None, 0
                gt = gpool.tile([P, maxcallcols, P], bf16, tag=f"g{qq}")
                ch = min(halfcols[(b, qq)], ccols)
                for hi_idx, (lo, hi) in enumerate(((0, ch), (ch, ccols))):
                    if hi > lo:
                        nc.gpsimd.dma_gather(
                            gt[:, lo:hi, :],
                            qtab[qq],
                            idx_sb[:, (c0 + lo) * 8 : (c0 + hi) * 8],
                            (hi - lo) * P,
                            (hi - lo) * P,
                            P,
                            single_packet=False,
                            queue_num=(2 * qq + hi_idx + b) % NQ,
                        )
                return gt, c0

            def load_ind(b, which):
                if which == "A":
                    ginc, goff, src, mx = gincA, goffA, indA_d, maxgA
                else:
                    ginc, goff, src, mx = gincB, goffB, indB_d, maxgB
                if ginc[b] == 0:
                    return None
                it = ipool.tile([P, mx * P], f8, tag=f"i{which}")
                nc.sync.dma_start(
                    out=it[:, : ginc[b] * P],
                    in_=src[:, goff[b] * P : (goff[b] + ginc[b]) * P],
                )
                return it

            def load_xot(b, selftabT):
                xot = tpool.tile([P, TG * P], bf16, tag="xot")
                nc.sync.dma_start(
                    out=xot[:],
                    in_=selftabT[:, b * TG * P : (b + 1) * TG * P],
                )
                return xot

            def agg_chain(b, t, cols, pos, goff, it, gts, bases, acc,
                          start, stop):
                n = len(cols)
                for ci, gc in enumerate(cols):
                    qq = next(
                        q2 for q2 in range(NQ)
                        if call_of[(b, q2)][1]
                        and call_of[(b, q2)][0] <= gc
                        < call_of[(b, q2)][0] + call_of[(b, q2)][1]
                    )
                    pp = (pos[b, t, gc] - goff[b]) * P
                    nc.tensor.matmul(
                        out=acc[:],
                        lhsT=gts[qq][:, gc - bases[qq], :],
                        rhs=it[:, pp : pp + P],
                        start=start and ci == 0,
                        stop=stop and ci == n - 1,
                    )

            pend = []

            def flush(n):
                while len(pend) > n:
                    pend.pop(0)()

            # ---------------- layer 1 -------------------------------------
            for b in range(NG):
                gts, bases = {}, {}
                for qq in range(NQ):
                    gt, c0 = gather(b, qq, qq, xq_aps)
                    if gt is not None:
                        gts[qq], bases[qq] = gt, c0
                iA = load_ind(b, "A")
                iB = load_ind(b, "B")
                xot = load_xot(b, xownT)
                hstage = stgpool.tile([P, TG, P], bf16, tag="hst")
                hTstage = stgpool.tile([P, TG * P], bf16, tag="hTst")
                for t in range(TG):
                    colsA, colsB = incA[b][t], incB[b][t]
                    ntot = len(colsA) + len(colsB)
                    acc = None
                    if ntot:
                        acc = ps_acc.tile([P, P], f32, tag="acc")
                        agg_chain(b, t, colsA, posA, goffA, iA, gts, bases,
                                  acc, True, not colsB)
                        agg_chain(b, t, colsB, posB, goffB, iB, gts, bases,
                                  acc, not colsA, True)

                    def hphase(b=b, t=t, acc=acc, xot=xot,
                               hstage=hstage, hTstage=hTstage):
                        tg = b * TG + t
                        aggTs = None
                        if acc is not None:
                            aggTs = apool.tile([P, P], bf16, tag="aggT")
                            nc.vector.tensor_mul(
                                out=aggTs[:], in0=acc[:],
                                in1=rbc[:, tg * P : (tg + 1) * P],
                            )
                        h = ps_h.tile([P, P], f32, tag="h")
                        if aggTs is not None:
                            nc.tensor.matmul(out=h[:], lhsT=aggTs[:],
                                             rhs=wl1[:], start=True,
                                             stop=False)
                        nc.tensor.matmul(
                            out=h[:], lhsT=xot[:, t * P : (t + 1) * P],
                            rhs=wr1[:], start=aggTs is None, stop=True,
                        )
                        nc.scalar.activation(
                            out=hstage[:, t, :], in_=h[:],
                            func=mybir.ActivationFunctionType.Relu,
                        )
                        hT = ps_hT.tile([P, P], f32, tag="hT")
                        if aggTs is not None:
                            nc.tensor.matmul(out=hT[:], lhsT=wl1[:],
                                             rhs=aggTs[:], start=True,
                                             stop=False)
                        nc.tensor.matmul(
                            out=hT[:], lhsT=wr1[:],
                            rhs=xot[:, t * P : (t + 1) * P],
                            start=aggTs is None, stop=True,
                        )
                        nc.scalar.activation(
                            out=hTstage[:, t * P : (t + 1) * P], in_=hT[:],
                            func=mybir.ActivationFunctionType.Relu,
                        )

                    pend.append(hphase)
                    flush(4)

                def blockout(b=b, hstage=hstage, hTstage=hTstage):
                    nc.sync.dma_start(
                        out=h_bounce[b * TG * P : (b + 1) * TG * P, :]
                        .rearrange("(t p) f -> p t f", p=P),
                        in_=hstage[:],
                    )
                    nc.sync.dma_start(
                        out=hT_bounce[:, b * TG * P : (b + 1) * TG * P],
                        in_=hTstage[:],
                    )

                pend.append(blockout)
            flush(0)

            # ---------------- the two half AllGathers ----------------------
            for i in range(2):
                nc.gpsimd.collective_compute(
                    "AllGather",
                    mybir.AluOpType.bypass,
                    replica_groups=[list(range(NCORES))],
                    ins=[h_bounce[i * HROWS : (i + 1) * HROWS, :]],
                    outs=[h_half[i][:]],
                )

            # ---------------- layer 2, phase X (src half 0 + self term) -----
            opart = aApool.tile([P, TPC * P], bf16, name="opart")
            for b in range(NG):
                gts, bases = {}, {}
                for j in range(2):
                    gt, c0 = gather(b, j, (2 * b + j) % NQ, hq_aps)
                    if gt is not None:
                        gts[j], bases[j] = gt, c0
                iA = load_ind(b, "A")
                xot = load_xot(b, hT_bounce)
                for t in range(TG):
                    colsA = incA[b][t]
                    acc = None
                    if colsA:
                        acc = ps_acc.tile([P, P], f32, tag="acc")
                        agg_chain(b, t, colsA, posA, goffA, iA, gts, bases,
                                  acc, True, True)

                    def xphase(b=b, t=t, acc=acc, xot=xot):
                        tg = b * TG + t
                        aggAs = None
                        if acc is not None:
                            aggAs = apool.tile([P, P], bf16, tag="aggT")
                            nc.vector.tensor_mul(
                                out=aggAs[:], in0=acc[:],
                                in1=rbc[:, tg * P : (tg + 1) * P],
                            )
                        op = ps_h.tile([P, P], f32, tag="h")
                        if aggAs is not None:
                            nc.tensor.matmul(
                                out=op[:], lhsT=wl2[:], rhs=aggAs[:],
                                start=True, stop=False,
                            )
                        nc.tensor.matmul(
                            out=op[:], lhsT=wr2[:],
                            rhs=xot[:, t * P : (t + 1) * P],
                            start=aggAs is None, stop=True,
                        )
                        nc.scalar.activation(
                            out=opart[:, tg * P : (tg + 1) * P], in_=op[:],
                            func=mybir.ActivationFunctionType.Copy,
                        )

                    pend.append(xphase)
                    flush(4)
            flush(0)

            # ---------------- layer 2, phase Y (src half 1 + output) --------
            for b in range(NG):
                gts, bases = {}, {}
                for j in range(2, 4):
                    gt, c0 = gather(b, j, (2 * b + j) % NQ, hq_aps)
                    if gt is not None:
                        gts[j], bases[j] = gt, c0
                iB = load_ind(b, "B")
                ostage = stgpool.tile([P, TG * P], bf16, tag="ost")
                for t in range(TG):
                    colsB = incB[b][t]
                    accB = None
                    if colsB:
                        accB = ps_acc.tile([P, P], f32, tag="acc")
                        agg_chain(b, t, colsB, posB, goffB, iB, gts, bases,
                                  accB, True, True)

                    def yphase(b=b, t=t, accB=accB, ostage=ostage):
                        tg = b * TG + t
                        if accB is not None:
                            aggBs = apool.tile([P, P], bf16, tag="aggB")
                            nc.vector.tensor_mul(
                                out=aggBs[:], in0=accB[:],
                                in1=rbc[:, tg * P : (tg + 1) * P],
                            )
                            o = ps_hT.tile([P, P], f32, tag="hT")
                            nc.tensor.matmul(
                                out=o[:], lhsT=wl2[:], rhs=aggBs[:],
                                start=True, stop=True,
                            )
                            nc.vector.tensor_add(
                                out=ostage[:, t * P : (t + 1) * P],
                                in0=opart[:, tg * P : (tg + 1) * P],
                                in1=o[:],
                            )
                        else:
                            nc.vector.tensor_copy(
                                out=ostage[:, t * P : (t + 1) * P],
                                in_=opart[:, tg * P : (tg + 1) * P],
                            )

                    pend.append(yphase)
                    flush(4)

                def oblock(b=b, ostage=ostage):
                    nc.sync.dma_start(
                        out=out_d[:, b * TG * P : (b + 1) * TG * P],
                        in_=ostage[:],
                    )

                pend.append(oblock)
            flush(0)

    return nc


def run(x, edge_index, W_l1, b_l1, W_r1, W_l2, b_l2, W_r2, trace=False):
    import ml_dtypes

    bf = ml_dtypes.bfloat16
    n_nodes = x.shape[0]
    assert n_nodes == NCORES * NPC

    rb, idx16, indA, indB, meta = _prep(np.asarray(edge_index))

    x = np.asarray(x, np.float32)
    # per-core padded slices, then relayout to [half, core, HROWS]
    xp = np.zeros((NCORES, NPC_PAD, P), bf)
    for c in range(NCORES):
        xp[c, :NPC] = x[c * NPC : (c + 1) * NPC]
    x_pad = np.ascontiguousarray(
        xp.reshape(NCORES, 2, HROWS, P).transpose(1, 0, 2, 3)
    ).reshape(2, NCORES * HROWS, P)

    skip_bias = not (np.any(np.asarray(b_l1)) or np.any(np.asarray(b_l2)))
    assert skip_bias, "nonzero SAGE biases not supported by this kernel"
    common = {
        "xh0": np.ascontiguousarray(x_pad[0]),
        "xh1": np.ascontiguousarray(x_pad[1]),
        "wl1": np.asarray(W_l1, bf),
        "wr1": np.asarray(W_r1, bf),
        "wl2": np.asarray(W_l2, bf),
        "wr2": np.asarray(W_r2, bf),
    }
    in_maps = []
    for c in range(NCORES):
        m = dict(common)
        m["xownT"] = np.ascontiguousarray(xp[c].T)
        m["idx16"] = idx16[c]
        m["indA"] = np.ascontiguousarray(indA[c])
        m["indB"] = np.ascontiguousarray(indB[c])
        m["rb"] = np.ascontiguousarray(rb[c])
        in_maps.append(m)

    nc = _build(meta, skip_bias)
    nc.finalize()

    from concourse.bass_utils import run_bass_kernel_spmd

    res = run_bass_kernel_spmd(nc, in_maps, list(range(NCORES)), trace=trace)
    out = np.empty((n_nodes, P), np.float32)
    for c in range(NCORES):
        out[c * NPC : (c + 1) * NPC] = (
            res.results[c]["out"][:, :NPC].astype(np.float32).T
        )
    return out, res


def kernel(x, edge_index, W_l1, b_l1, W_r1, W_l2, b_l2, W_r2):
    out, _ = run(x, edge_index, W_l1, b_l1, W_r1, W_l2, b_l2, W_r2)
    return out


# revision 19
# speedup vs baseline: 1.0144x; 1.0144x over previous
"""Two-layer GraphSAGE (mean aggregation) on 8 Trainium2 NeuronCores.

Strategy (dst-partitioned, graph/data parallel):
- Nodes partitioned by destination across 8 cores (12500 each, padded to
  12544 = 98*128). The padded global row space is [half, core, 6272]; the
  between-layer AllGather is two half-collectives whose outputs are the four
  25088-row gather tables (quarter q = (half q//2, core group q%2), all
  int16-addressable).
- Edges bucketed per core by (group of 14 dst tiles, src quarter), sorted by
  dst tile. Source rows are fetched with large SWDGE dma_gather calls spread
  over the 4 SWDGE queues (queue q -> Q7 core pair 2q/2q+1) so descriptor
  generation proceeds 4-way parallel. Slot layout is identical on every core
  (capacity = max count over cores; pad slots gather row 0 with a zero
  indicator row) keeping the SPMD program uniform.
- Aggregation: 0/1 indicator tiles (slot -> dst one-hots) are host data,
  precomputed in fp8e4m3 and DMA'd in (HWDGE), used directly as the moving
  operand of the PE accumulation matmuls (bf16 lhsT x fp8 rhs). Indicators
  are A/B-split (A = src half 0, B = half 1) so layer 2 runs an X phase
  (A-side partial aggregation after the first half-AllGather) and a Y phase
  (B side + output) gated only on the second half-AllGather.
- Mean normalization is a single DVE multiply with a resident recip-broadcast
  tile; h = relu(aggTs @ W_l + selfT @ W_r) accumulates in one PSUM group.
  No dma_start_transpose anywhere (it serializes against collectives): x
  arrives host-transposed, layer 1 emits h^T via a second PSUM chain
  (W^T @ aggTs), and layer 2 produces the output transposed ([feat, node]);
  the host transposes it back. h/h^T/out DMA writes are batched per block.

kernel(**inputs) -> np.ndarray takes FULL inputs, returns FULL [100000, 128]
float32 output; all sharding happens inside.
"""

import numpy as np

P = 128
NCORES = 8
NPC = 12500
TPC = 98
NPC_PAD = TPC * P            # 12544
NALL = NCORES * NPC_PAD      # 100352
NQ = 4                       # src quarters (= gather tables)
HROWS = NPC_PAD // 2         # 6272 rows per half per core
QROWS = NALL // NQ           # 25088 rows per gather table (< int16 range)
TG = 14                      # dst tiles per group
NG = TPC // TG               # 7 groups (= gather blocks)


def _prep(edge_index):
    src = edge_index[0].astype(np.int64)
    dst = edge_index[1].astype(np.int64)
    core = dst // NPC
    loc = dst % NPC
    tl = loc // P
    off = loc % P
    g = tl // TG
    tl_loc = tl % TG
    # padded global row space: [half, core, HROWS]
    sloc = src % NPC
    half = sloc // HROWS
    srcpad = half * (NCORES * HROWS) + (src // NPC) * HROWS + sloc % HROWS
    q = srcpad // QROWS
    qrow = srcpad % QROWS

    key = ((core * NG + g) * NQ + q) * TG + tl_loc
    cnt = np.bincount(key, minlength=NCORES * NG * NQ * TG).reshape(
        NCORES, NG, NQ, TG
    )
    cap = cnt.max(axis=0)
    scum = np.zeros((NG, NQ, TG + 1), np.int64)
    np.cumsum(cap, axis=2, out=scum[:, :, 1:])
    segcols = -(-scum[:, :, TG] // P)

    # one gather call per (block=group, quarter); columns packed q-major
    colbase = np.zeros((NG, NQ), np.int64)
    calls = []                                  # (g, q, col0, callcols)
    ncols = 0
    for gg in range(NG):
        for qq in range(NQ):
            colbase[gg, qq] = ncols
            ncols += int(segcols[gg, qq])
            calls.append((gg, qq, int(colbase[gg, qq]), int(segcols[gg, qq])))

    order = np.lexsort((tl_loc, q, g, core))
    sk = key[order]
    first = np.r_[True, sk[1:] != sk[:-1]]
    idx_of_first = np.where(first)[0]
    grp_id = np.cumsum(first) - 1
    rank = np.arange(len(sk)) - idx_of_first[grp_id]
    go, qo, to, co = g[order], q[order], tl_loc[order], core[order]
    slot = scum[go, qo, to] + rank
    gcol = colbase[go, qo] + slot // P
    prow = slot % P

    cnt_dst = np.bincount(dst, minlength=NCORES * NPC).astype(np.float64)
    recip_dst = (1.0 / np.maximum(cnt_dst, 1.0)).astype(np.float32)

    idx16 = np.zeros((NCORES, 16, ncols * 8), np.int16)
    callslot = (gcol - colbase[go, qo]) * P + prow
    callb = colbase[go, qo] * 8
    idx16[co, callslot % 16, callb + callslot // 16] = qrow[order].astype(
        np.int16
    )
    idx16 = np.ascontiguousarray(np.tile(idx16, (1, 8, 1)))

    # incidence lists: per (g, t) the columns intersecting tile t, split into
    # A (src half 0: quarters 0-1) and B (half 1: quarters 2-3)
    incA = [[[] for _ in range(TG)] for _ in range(NG)]
    incB = [[[] for _ in range(TG)] for _ in range(NG)]
    posA = np.full((NG, TG, ncols), -1, np.int64)
    posB = np.full((NG, TG, ncols), -1, np.int64)
    nA = nB = 0
    for gg in range(NG):
        for t in range(TG):
            for qq in range(NQ):
                c = cap[gg, qq, t]
                if c == 0:
                    continue
                s0 = scum[gg, qq, t]
                for cc in range(s0 // P, -(-(s0 + c) // P)):
                    col = int(colbase[gg, qq] + cc)
                    if qq < 2:
                        if posA[gg, t, col] < 0:
                            posA[gg, t, col] = nA
                            incA[gg][t].append(col)
                            nA += 1
                    else:
                        if posB[gg, t, col] < 0:
                            posB[gg, t, col] = nB
                            incB[gg][t].append(col)
                            nB += 1

    import ml_dtypes

    f8 = ml_dtypes.float8_e4m3
    indA = np.zeros((NCORES, P, nA * P), f8)
    indB = np.zeros((NCORES, P, nB * P), f8)
    oo = off[order]
    isA = qo < 2
    pA = posA[go[isA], to[isA], gcol[isA]]
    assert (pA >= 0).all()
    indA[co[isA], prow[isA], pA * P + oo[isA]] = 1.0
    isB = ~isA
    pB = posB[go[isB], to[isB], gcol[isB]]
    assert (pB >= 0).all()
    indB[co[isB], prow[isB], pB * P + oo[isB]] = 1.0

    # recip broadcast tiles: [P, NPC_PAD] bf16, every partition row equal
    bf = ml_dtypes.bfloat16
    rb = np.zeros((NCORES, P, NPC_PAD), bf)
    for c in range(NCORES):
        r = np.zeros(NPC_PAD, np.float32)
        r[:NPC] = recip_dst[c * NPC : (c + 1) * NPC]
        rb[c] = np.broadcast_to(r.astype(bf), (P, NPC_PAD))

    # per-group offsets into the A/B indicator tensors
    goffA = [min((posA[gg, t, c] for t in range(TG) for c in incA[gg][t]),
                 default=0) for gg in range(NG)]
    gincA = [sum(len(incA[gg][t]) for t in range(TG)) for gg in range(NG)]
    goffB = [min((posB[gg, t, c] for t in range(TG) for c in incB[gg][t]),
                 default=0) for gg in range(NG)]
    gincB = [sum(len(incB[gg][t]) for t in range(TG)) for gg in range(NG)]

    halfcols = {}
    for gg in range(NG):
        for qq in range(NQ):
            halfcols[(gg, qq)] = int(-(-scum[gg, qq, TG // 2] // P))

    meta = dict(
        halfcols=halfcols,
        calls=calls, ncols=ncols,
        incA=incA, incB=incB, posA=posA, posB=posB,
        goffA=goffA, gincA=gincA, goffB=goffB, gincB=gincB,
        nA=nA, nB=nB,
    )
    return rb, idx16, indA, indB, meta


def _build(meta, skip_bias):
    from concourse import bacc, bass, mybir, tile

    bf16 = mybir.dt.bfloat16
    f8 = mybir.dt.float8e4
    f32 = mybir.dt.float32
    i16 = mybir.dt.int16

    calls = meta["calls"]
    halfcols = meta["halfcols"]
    ncols = meta["ncols"]
    incA, incB = meta["incA"], meta["incB"]
    posA, posB = meta["posA"], meta["posB"]
    goffA, gincA = meta["goffA"], meta["gincA"]
    goffB, gincB = meta["goffB"], meta["gincB"]
    nA, nB = meta["nA"], meta["nB"]

    nc = bacc.Bacc(
        "TRN2", target_bir_lowering=False, debug=False, num_devices=NCORES,
        num_swdge_queues=4,
    )

    xh = [
        nc.declare_dram_parameter(f"xh{i}", [2 * QROWS, P], bf16,
                                  isOutput=False)
        for i in range(2)
    ]
    xownT = nc.declare_dram_parameter("xownT", [P, NPC_PAD], bf16, isOutput=False)
    idx_d = nc.declare_dram_parameter("idx16", [P, ncols * 8], i16, isOutput=False)
    indA_d = nc.declare_dram_parameter("indA", [P, nA * P], f8, isOutput=False)
    indB_d = nc.declare_dram_parameter("indB", [P, nB * P], f8, isOutput=False)
    rb_d = nc.declare_dram_parameter("rb", [P, NPC_PAD], bf16, isOutput=False)
    wl1_d = nc.declare_dram_parameter("wl1", [P, P], bf16, isOutput=False)
    wr1_d = nc.declare_dram_parameter("wr1", [P, P], bf16, isOutput=False)
    wl2_d = nc.declare_dram_parameter("wl2", [P, P], bf16, isOutput=False)
    wr2_d = nc.declare_dram_parameter("wr2", [P, P], bf16, isOutput=False)
    out_d = nc.declare_dram_parameter("out", [P, NPC_PAD], bf16, isOutput=True)

    maxcallcols = max(c[3] for c in calls)
    maxgA = max(gincA) if max(gincA) else 1
    maxgB = max(gincB) if max(gincB) else 1
    call_of = {}
    for (gg, qq, c0, ccols) in calls:
        call_of[(gg, qq)] = (c0, ccols)

    with tile.TileContext(nc) as tc:
        with (
            tc.tile_pool(name="const", bufs=1) as cpool,
            tc.tile_pool(name="gath", bufs=2) as gpool,
            tc.tile_pool(name="indp", bufs=2) as ipool,
            tc.tile_pool(name="xot", bufs=3) as tpool,
            tc.tile_pool(name="aggp", bufs=8) as apool,
            tc.tile_pool(name="aggA", bufs=1) as aApool,
            tc.tile_pool(name="stg", bufs=2) as stgpool,
            tc.tile_pool(name="psacc", bufs=4, space="PSUM") as ps_acc,
            tc.tile_pool(name="psh", bufs=2, space="PSUM") as ps_h,
            tc.tile_pool(name="psht", bufs=2, space="PSUM") as ps_hT,
            tc.tile_pool(name="dram", bufs=1, space="DRAM") as dpool,
        ):
            def cload(dram_ap, shape, dtype, name):
                t = cpool.tile(shape, dtype, name=name)
                nc.sync.dma_start(out=t[:], in_=dram_ap)
                return t

            idx_sb = cload(idx_d[:], [P, ncols * 8], i16, "idx16")
            wl1 = cload(wl1_d[:], [P, P], bf16, "wl1")
            wr1 = cload(wr1_d[:], [P, P], bf16, "wr1")
            wl2 = cload(wl2_d[:], [P, P], bf16, "wl2")
            rbc = cload(rb_d[:], [P, NPC_PAD], bf16, "rb")
            wr2 = cload(wr2_d[:], [P, P], bf16, "wr2")

            h_bounce = dpool.tile([NPC_PAD, P], bf16, name="h_bounce")
            hT_bounce = dpool.tile([P, NPC_PAD], bf16, name="hT_bounce")
            h_half = [
                dpool.tile([2 * QROWS, P], bf16, name=f"h_half{i}",
                           addr_space="Shared")
                for i in range(2)
            ]
            xq_aps = [
                xh[0][0:QROWS, :], xh[0][QROWS : 2 * QROWS, :],
                xh[1][0:QROWS, :], xh[1][QROWS : 2 * QROWS, :],
            ]
            hq_aps = [
                h_half[0][0:QROWS, :], h_half[0][QROWS : 2 * QROWS, :],
                h_half[1][0:QROWS, :], h_half[1][QROWS : 2 * QROWS, :],
            ]

            def gather(b, qq, queue, qtab):
                c0, ccols = call_of[(b, qq)]
                if ccols == 0:
                    return None, 0
                gt = gpool.tile([P, maxcallcols, P], bf16, tag=f"g{qq}")
                ch = min(halfcols[(b, qq)], ccols)
                for lo, hi in ((0, ch), (ch, ccols)):
                    if hi > lo:
                        nc.gpsimd.dma_gather(
                            gt[:, lo:hi, :],
                            qtab[qq],
                            idx_sb[:, (c0 + lo) * 8 : (c0 + hi) * 8],
                            (hi - lo) * P,
                            (hi - lo) * P,
                            P,
                            single_packet=False,
                            queue_num=queue,
                        )
                return gt, c0

            def load_ind(b, which):
                if which == "A":
                    ginc, goff, src, mx = gincA, goffA, indA_d, maxgA
                else:
                    ginc, goff, src, mx = gincB, goffB, indB_d, maxgB
                if ginc[b] == 0:
                    return None
                it = ipool.tile([P, mx * P], f8, tag=f"i{which}")
                nc.sync.dma_start(
                    out=it[:, : ginc[b] * P],
                    in_=src[:, goff[b] * P : (goff[b] + ginc[b]) * P],
                )
                return it

            def load_xot(b, selftabT):
                xot = tpool.tile([P, TG * P], bf16, tag="xot")
                nc.sync.dma_start(
                    out=xot[:],
                    in_=selftabT[:, b * TG * P : (b + 1) * TG * P],
                )
                return xot

            def agg_chain(b, t, cols, pos, goff, it, gts, bases, acc,
                          start, stop):
                n = len(cols)
                for ci, gc in enumerate(cols):
                    qq = next(
                        q2 for q2 in range(NQ)
                        if call_of[(b, q2)][1]
                        and call_of[(b, q2)][0] <= gc
                        < call_of[(b, q2)][0] + call_of[(b, q2)][1]
                    )
                    pp = (pos[b, t, gc] - goff[b]) * P
                    nc.tensor.matmul(
                        out=acc[:],
                        lhsT=gts[qq][:, gc - bases[qq], :],
                        rhs=it[:, pp : pp + P],
                        start=start and ci == 0,
                        stop=stop and ci == n - 1,
                    )

            pend = []

            def flush(n):
                while len(pend) > n:
                    pend.pop(0)()

            # ---------------- layer 1 -------------------------------------
            for b in range(NG):
                gts, bases = {}, {}
                for qq in range(NQ):
                    gt, c0 = gather(b, qq, qq, xq_aps)
                    if gt is not None:
                        gts[qq], bases[qq] = gt, c0
                iA = load_ind(b, "A")
                iB = load_ind(b, "B")
                xot = load_xot(b, xownT)
                hstage = stgpool.tile([P, TG, P], bf16, tag="hst")
                hTstage = stgpool.tile([P, TG * P], bf16, tag="hTst")
                for t in range(TG):
                    colsA, colsB = incA[b][t], incB[b][t]
                    ntot = len(colsA) + len(colsB)
                    acc = None
                    if ntot:
                        acc = ps_acc.tile([P, P], f32, tag="acc")
                        agg_chain(b, t, colsA, posA, goffA, iA, gts, bases,
                                  acc, True, not colsB)
                        agg_chain(b, t, colsB, posB, goffB, iB, gts, bases,
                                  acc, not colsA, True)

                    def hphase(b=b, t=t, acc=acc, xot=xot,
                               hstage=hstage, hTstage=hTstage):
                        tg = b * TG + t
                        aggTs = None
                        if acc is not None:
                            aggTs = apool.tile([P, P], bf16, tag="aggT")
                            nc.vector.tensor_mul(
                                out=aggTs[:], in0=acc[:],
                                in1=rbc[:, tg * P : (tg + 1) * P],
                            )
                        h = ps_h.tile([P, P], f32, tag="h")
                        if aggTs is not None:
                            nc.tensor.matmul(out=h[:], lhsT=aggTs[:],
                                             rhs=wl1[:], start=True,
                                             stop=False)
                        nc.tensor.matmul(
                            out=h[:], lhsT=xot[:, t * P : (t + 1) * P],
                            rhs=wr1[:], start=aggTs is None, stop=True,
                        )
                        nc.scalar.activation(
                            out=hstage[:, t, :], in_=h[:],
                            func=mybir.ActivationFunctionType.Relu,
                        )
                        hT = ps_hT.tile([P, P], f32, tag="hT")
                        if aggTs is not None:
                            nc.tensor.matmul(out=hT[:], lhsT=wl1[:],
                                             rhs=aggTs[:], start=True,
                                             stop=False)
                        nc.tensor.matmul(
                            out=hT[:], lhsT=wr1[:],
                            rhs=xot[:, t * P : (t + 1) * P],
                            start=aggTs is None, stop=True,
                        )
                        nc.scalar.activation(
                            out=hTstage[:, t * P : (t + 1) * P], in_=hT[:],
                            func=mybir.ActivationFunctionType.Relu,
                        )

                    pend.append(hphase)
                    flush(4)

                def blockout(b=b, hstage=hstage, hTstage=hTstage):
                    nc.sync.dma_start(
                        out=h_bounce[b * TG * P : (b + 1) * TG * P, :]
                        .rearrange("(t p) f -> p t f", p=P),
                        in_=hstage[:],
                    )
                    nc.sync.dma_start(
                        out=hT_bounce[:, b * TG * P : (b + 1) * TG * P],
                        in_=hTstage[:],
                    )

                pend.append(blockout)
            flush(0)

            # ---------------- the two half AllGathers ----------------------
            for i in range(2):
                nc.gpsimd.collective_compute(
                    "AllGather",
                    mybir.AluOpType.bypass,
                    replica_groups=[list(range(NCORES))],
                    ins=[h_bounce[i * HROWS : (i + 1) * HROWS, :]],
                    outs=[h_half[i][:]],
                )

            # ---------------- layer 2, phase X (src half 0 + self term) -----
            opart = aApool.tile([P, TPC * P], bf16, name="opart")
            for b in range(NG):
                gts, bases = {}, {}
                for j in range(2):
                    gt, c0 = gather(b, j, (2 * b + j) % NQ, hq_aps)
                    if gt is not None:
                        gts[j], bases[j] = gt, c0
                iA = load_ind(b, "A")
                xot = load_xot(b, hT_bounce)
                for t in range(TG):
                    colsA = incA[b][t]
                    acc = None
                    if colsA:
                        acc = ps_acc.tile([P, P], f32, tag="acc")
                        agg_chain(b, t, colsA, posA, goffA, iA, gts, bases,
                                  acc, True, True)

                    def xphase(b=b, t=t, acc=acc, xot=xot):
                        tg = b * TG + t
                        aggAs = None
                        if acc is not None:
                            aggAs = apool.tile([P, P], bf16, tag="aggT")
                            nc.vector.tensor_mul(
                                out=aggAs[:], in0=acc[:],
                                in1=rbc[:, tg * P : (tg + 1) * P],
                            )
                        op = ps_h.tile([P, P], f32, tag="h")
                        if aggAs is not None:
                            nc.tensor.matmul(
                                out=op[:], lhsT=wl2[:], rhs=aggAs[:],
                                start=True, stop=False,
                            )
                        nc.tensor.matmul(
                            out=op[:], lhsT=wr2[:],
                            rhs=xot[:, t * P : (t + 1) * P],
                            start=aggAs is None, stop=True,
                        )
                        nc.scalar.activation(
                            out=opart[:, tg * P : (tg + 1) * P], in_=op[:],
                            func=mybir.ActivationFunctionType.Copy,
                        )

                    pend.append(xphase)
                    flush(4)
            flush(0)

            # ---------------- layer 2, phase Y (src half 1 + output) --------
            for b in range(NG):
                gts, bases = {}, {}
                for j in range(2, 4):
                    gt, c0 = gather(b, j, (2 * b + j) % NQ, hq_aps)
                    if gt is not None:
                        gts[j], bases[j] = gt, c0
                iB = load_ind(b, "B")
                ostage = stgpool.tile([P, TG * P], bf16, tag="ost")
                for t in range(TG):
                    colsB = incB[b][t]
                    accB = None
                    if colsB:
                        accB = ps_acc.tile([P, P], f32, tag="acc")
                        agg_chain(b, t, colsB, posB, goffB, iB, gts, bases,
                                  accB, True, True)

                    def yphase(b=b, t=t, accB=accB, ostage=ostage):
                        tg = b * TG + t
                        if accB is not None:
                            aggBs = apool.tile([P, P], bf16, tag="aggB")
                            nc.vector.tensor_mul(
                                out=aggBs[:], in0=accB[:],
                                in1=rbc[:, tg * P : (tg + 1) * P],
                            )
                            o = ps_hT.tile([P, P], f32, tag="hT")
                            nc.tensor.matmul(
                                out=o[:], lhsT=wl2[:], rhs=aggBs[:],
                                start=True, stop=True,
                            )
                            nc.vector.tensor_add(
                                out=ostage[:, t * P : (t + 1) * P],
                                in0=opart[:, tg * P : (tg + 1) * P],
                                in1=o[:],
                            )
                        else:
                            nc.vector.tensor_copy(
                                out=ostage[:, t * P : (t + 1) * P],
                                in_=opart[:, tg * P : (tg + 1) * P],
                            )

                    pend.append(yphase)
                    flush(4)

                def oblock(b=b, ostage=ostage):
                    nc.sync.dma_start(
                        out=out_d[:, b * TG * P : (b + 1) * TG * P],
                        in_=ostage[:],
                    )

                pend.append(oblock)
            flush(0)

    return nc


def run(x, edge_index, W_l1, b_l1, W_r1, W_l2, b_l2, W_r2, trace=False):
    import ml_dtypes

    bf = ml_dtypes.bfloat16
    n_nodes = x.shape[0]
    assert n_nodes == NCORES * NPC

    rb, idx16, indA, indB, meta = _prep(np.asarray(edge_index))

    x = np.asarray(x, np.float32)
    # per-core padded slices, then relayout to [half, core, HROWS]
    xp = np.zeros((NCORES, NPC_PAD, P), bf)
    for c in range(NCORES):
        xp[c, :NPC] = x[c * NPC : (c + 1) * NPC]
    x_pad = np.ascontiguousarray(
        xp.reshape(NCORES, 2, HROWS, P).transpose(1, 0, 2, 3)
    ).reshape(2, NCORES * HROWS, P)

    skip_bias = not (np.any(np.asarray(b_l1)) or np.any(np.asarray(b_l2)))
    assert skip_bias, "nonzero SAGE biases not supported by this kernel"
    common = {
        "xh0": np.ascontiguousarray(x_pad[0]),
        "xh1": np.ascontiguousarray(x_pad[1]),
        "wl1": np.asarray(W_l1, bf),
        "wr1": np.asarray(W_r1, bf),
        "wl2": np.asarray(W_l2, bf),
        "wr2": np.asarray(W_r2, bf),
    }
    in_maps = []
    for c in range(NCORES):
        m = dict(common)
        m["xownT"] = np.ascontiguousarray(xp[c].T)
        m["idx16"] = idx16[c]
        m["indA"] = np.ascontiguousarray(indA[c])
        m["indB"] = np.ascontiguousarray(indB[c])
        m["rb"] = np.ascontiguousarray(rb[c])
        in_maps.append(m)

    nc = _build(meta, skip_bias)
    nc.finalize()

    from concourse.bass_utils import run_bass_kernel_spmd

    res = run_bass_kernel_spmd(nc, in_maps, list(range(NCORES)), trace=trace)
    out = np.empty((n_nodes, P), np.float32)
    for c in range(NCORES):
        out[c * NPC : (c + 1) * NPC] = (
            res.results[c]["out"][:, :NPC].astype(np.float32).T
        )
    return out, res


def kernel(x, edge_index, W_l1, b_l1, W_r1, W_l2, b_l2, W_r2):
    out, _ = run(x, edge_index, W_l1, b_l1, W_r1, W_l2, b_l2, W_r2)
    return out
